# revision 35
# baseline (speedup 1.0000x reference)
"""GRU autoencoder Trainium2 kernel.

Data-parallel over batch: 8 cores x 64 rows. Per core, the recurrence keeps
the hidden state TRANSPOSED in SBUF (hT[klo, 64*khi+b] = h[b, 128*khi+klo])
so it can be the stationary matmul operand directly. Gates are computed as
h @ W.T with fp32r matmuls (M=64, N=512) accumulating in PSUM on top of K=1
bias-seed matmuls; the seeds sit in four distinct PE row groups so adjacent
seeds stream concurrently through the array. The z/n gates are folded onto
both partition halves so their transposes back to hT layout (transpose-mode
identity matmuls) run as concurrent row-group pairs into separate PSUM
banks. The hidden update uses hT' = zT*hT - (zT-1)*nT with the zT terms
computed while the tanh path is still in flight. Next-step input-gate
matmuls and decoder z-outputs are emitted during the elementwise tail so
the PE never drains; in/hn psums are seeded mid-step to cut PSUM-bank
pressure.
"""
import os
import sys
import types

import numpy as np

import concourse.bass as bass
import concourse.mybir as mybir
import concourse.tile as tile
from concourse import bass_utils

F32 = mybir.dt.float32
F32R = mybir.dt.float32r
AF = mybir.ActivationFunctionType
OP = mybir.AluOpType

N_CORES = 8
B, T, I, H = 512, 128, 512, 1024
BL = B // N_CORES  # 64


# ---------------------------------------------------------------- fixups
_CTRL_OPCODES = {"Drain", "NoOp", "EventSemaphore", "AllEngineBarrier", "Halt"}


def _split_multi_waits(nc, max_waits=1):
    """This walrus build allows only one sync-wait per instruction; hoist
    excess waits onto preceding NoOps (same engine, so semantics hold)."""
    for f in nc.m.functions:
        for blk in f.blocks:
            insts = blk.instructions
            if not any(
                i.sync_info is not None
                and i.sync_info.on_wait
                and len(i.sync_info.on_wait) > max_waits
                for i in insts
            ):
                continue
            new = []
            for inst in insts:
                si = inst.sync_info
                if si is not None and si.on_wait and len(si.on_wait) > max_waits:
                    waits = list(si.on_wait)
                    extra, keep = waits[:-max_waits], waits[-max_waits:]
                    for cs in range(0, len(extra), max_waits):
                        nop = mybir.InstNoOp(
                            name=nc.get_next_instruction_name(),
                            engine=inst.engine,
                            ins=[],
                            outs=[],
                            sync_info=mybir.SyncInfo(
                                on_wait=extra[cs : cs + max_waits], on_update=[]
                            ),
                        )
                        nc.register_instruction(nop)
                        new.append(nop)
                    si.on_wait = keep
                new.append(inst)
            insts[:] = new


def _install_ntff_hook():
    if "antenv.axon_hooks" in sys.modules:
        return True
    mod = types.ModuleType("antenv.axon_hooks")
    state = {"hook": None}
    mod.set_axon_ntff_profile_hook = lambda h: state.__setitem__("hook", h)
    mod.get_axon_ntff_profile_hook = lambda: state["hook"]
    sys.modules["antenv.axon_hooks"] = mod
    try:
        import antenv

        antenv.axon_hooks = mod
        from trn_agent_boot.trn_boot import _ntff_profile_via_ctypes

        hook = _ntff_profile_via_ctypes("/opt/axon/libaxon_pjrt.so")
        if hook is None:
            return False
        mod.set_axon_ntff_profile_hook(hook)
        return True
    except Exception:
        return False


# ---------------------------------------------------------------- program
def build_nc(n_steps=T):
    nc = bass.Bass("TRN2", target_bir_lowering=False, debug=False, num_devices=N_CORES)

    xT_d = nc.dram_tensor("xT", [n_steps, 128, 4, BL], F32R, kind="ExternalInput").ap()
    wih_d = nc.dram_tensor("wihT", [4, 128, 3 * H], F32R, kind="ExternalInput").ap()
    whh_d = nc.dram_tensor("whhT", [8, 128, 3 * H], F32R, kind="ExternalInput").ap()
    wcb_d = nc.dram_tensor("wcombT", [8, 128, 4 * H], F32R, kind="ExternalInput").ap()
    wz_d = nc.dram_tensor("wzT", [8, 128, I], F32R, kind="ExternalInput").ap()
    br_d = nc.dram_tensor("brows", [128, 2048], F32R, kind="ExternalInput").ap()
    bz_d = nc.dram_tensor("bzt", [64, I], F32, kind="ExternalInput").ap()
    on_d = nc.dram_tensor("ones", [128, 64], F32R, kind="ExternalInput").ap()
    id_d = nc.dram_tensor("iden", [128, 64], F32R, kind="ExternalInput").ap()
    h0_d = nc.dram_tensor("h0T", [128, 512], F32R, kind="ExternalInput").ap()
    z_d = nc.dram_tensor("z", [BL, n_steps, I], F32, kind="ExternalOutput").ap()

    with tile.TileContext(nc) as tc:
        with (
            tc.tile_pool(name="cst", bufs=1) as cst,
            tc.tile_pool(name="hst", bufs=3) as hst,
            tc.tile_pool(name="xts", bufs=5) as xts,
            tc.tile_pool(name="gsb", bufs=2) as gsb,
            tc.tile_pool(name="tmp", bufs=2) as tmpp,
            tc.tile_pool(name="zo", bufs=2) as zop,
            tc.tile_pool(name="ps", bufs=8, space="PSUM") as ps,
        ):
            brows = cst.tile([128, 2048], F32R)
            nc.sync.dma_start(brows[:], br_d[:])
            bzt = cst.tile([64, I], F32)
            nc.sync.dma_start(bzt[:], bz_d[:])
            ones = cst.tile([128, 64], F32R)
            nc.sync.dma_start(ones[:], on_d[:])
            iden = cst.tile([128, 64], F32R)
            nc.sync.dma_start(iden[:], id_d[:])
            hT = hst.tile([128, 512], F32R, tag="h")
            nc.sync.dma_start(hT[:], h0_d[:])

            def seed(pt, brow_ap, one_ap, bp):
                nc.tensor.matmul(pt[:], one_ap, brow_ap, start=True, stop=False,
                                 tile_position=(bp, 0))

            def alloc_seed_pair(nm, t, enc):
                """Allocate+bias-seed one gate pair (2 psum tiles).

                The two seeds of a pair sit in different PE row groups (and
                z/in vs r/hn pairs in different groups again), so adjacent
                seed matmuls stream concurrently in the array."""
                cbase = 0 if enc else 1024
                rows, c0 = {
                    "pz": ((0, 32), cbase), "pr": ((64, 96), cbase),
                    "pin": ((0, 32), cbase + 512), "phn": ((64, 96), cbase + 512),
                }[nm]
                tiles = [ps.tile([64, 512], F32, tag="ps", name=f"{nm}{i}_{t}")
                         for i in range(2)]
                for nt in range(2):
                    row = rows[nt]
                    seed(tiles[nt], brows[row : row + 1, c0 : c0 + 512],
                         ones[row : row + 1, :], row)
                return tiles

            def emit_gi_zr(g, xt):
                for tiles, c0 in ((g["pz"], 1024), (g["pr"], 0)):
                    for nt in range(2):
                        c = c0 + 512 * nt
                        for k in range(4):
                            nc.tensor.matmul(
                                tiles[nt][:], xt[:, k, :], wih[:, k, c : c + 512],
                                start=False, stop=False,
                            )

            def emit_gi_in(g, xt):
                for nt in range(2):
                    c = 2048 + 512 * nt
                    for k in range(4):
                        nc.tensor.matmul(
                            g["pin"][nt][:], xt[:, k, :], wih[:, k, c : c + 512],
                            start=False, stop=(k == 3),
                        )

            def emit_gh(g, w, cols):
                """Recurrent gate matmuls reading hT: order z, r, (in), hn."""
                for nm, c0 in cols:
                    for nt in range(2):
                        c = c0 + 512 * nt
                        for k in range(8):
                            nc.tensor.matmul(
                                g[nm][nt][:],
                                hT[:, 64 * k : 64 * k + 64],
                                w[:, k, c : c + 512],
                                start=False, stop=(k == 7),
                            )

            def emit_zfill(src_hT, t_out):
                pzo = ps.tile([64, 512], F32, tag="ps", name=f"pzo{t_out}")
                for j in range(8):
                    nc.tensor.matmul(
                        pzo[:], src_hT[:, 64 * j : 64 * j + 64], wz[:, j, :],
                        start=(j == 0), stop=(j == 7),
                    )
                zo_sb = zop.tile([64, 512], F32, tag="zo", name=f"zo{t_out}")
                nc.vector.tensor_add(zo_sb[:], pzo[:], bzt[:])
                nc.sync.dma_start(z_d[:, t_out, :], zo_sb[:])

            def step_tail(t, g, filler):
                """sigmoids, transposes, n-chain, h-update; filler() emits
                next-step PE work between zT and nT transposes."""
                nonlocal hT
                # z folded onto both partition halves so its transposes run
                # as concurrent row-group pairs
                z_f = gsb.tile([128, 512], F32R, tag="z", name=f"z{t}")
                for nt in range(2):
                    nc.scalar.activation(z_f[64 * nt : 64 * nt + 64, :],
                                         g["pz"][nt][:], AF.Sigmoid)
                # two half tiles in separate banks: the row-group pairs may
                # not write one PSUM bank concurrently
                pzT_h = [ps.tile([128, 256], F32R, tag="ps", name=f"pzT{t}_{i}")
                         for i in range(2)]
                for j in range(4):
                    nc.tensor.transpose(
                        pzT_h[0][:, 64 * j : 64 * j + 64],
                        z_f[0:64, 128 * j : 128 * j + 128],
                        iden[0:64, :],
                    )
                    nc.tensor.transpose(
                        pzT_h[1][:, 64 * j : 64 * j + 64],
                        z_f[64:128, 128 * j : 128 * j + 128],
                        iden[64:128, :],
                    )
                r_sb = gsb.tile([64, 1024], F32, tag="r", name=f"r{t}")
                for nt in range(2):
                    nc.scalar.activation(r_sb[:, 512 * nt : 512 * nt + 512],
                                         g["pr"][nt][:], AF.Sigmoid)

                if filler is not None:
                    filler()

                # u = zT * hT can run while the n-path (tanh/transpose) works
                u_t = tmpp.tile([128, 512], F32, tag="u", name=f"u{t}")

                # n = tanh(in + r*hn), folded onto partition halves like z so
                # the transposes run as concurrent row-group pairs
                n_f = gsb.tile([128, 512], F32R, tag="n", name=f"n{t}")
                pnT_h = [ps.tile([128, 256], F32R, tag="ps", name=f"pnT{t}_{i}")
                         for i in range(2)]
                for nt in range(2):
                    t1 = tmpp.tile([64, 512], F32, tag="t1", name=f"t1_{t}_{nt}")
                    nc.vector.tensor_mul(t1[:], r_sb[:, 512 * nt : 512 * nt + 512],
                                         g["phn"][nt][:])
                    nc.vector.tensor_add(t1[:], t1[:], g["pin"][nt][:])
                    nc.scalar.activation(n_f[64 * nt : 64 * nt + 64, :],
                                         t1[:], AF.Tanh)
                for j in range(4):
                    nc.tensor.transpose(
                        pnT_h[0][:, 64 * j : 64 * j + 64],
                        n_f[0:64, 128 * j : 128 * j + 128],
                        iden[0:64, :],
                    )
                    nc.tensor.transpose(
                        pnT_h[1][:, 64 * j : 64 * j + 64],
                        n_f[64:128, 128 * j : 128 * j + 128],
                        iden[64:128, :],
                    )

                # hT' = zT*hT + (1-zT)*nT as u - (zT-1)*nT; u and the zT sbuf
                # stage only need zT, so they overlap the tanh/nT path.
                zT_sb = tmpp.tile([128, 512], F32, tag="zts", name=f"zts{t}")
                for hh in range(2):
                    s = slice(256 * hh, 256 * hh + 256)
                    nc.vector.tensor_mul(u_t[:, s], pzT_h[hh][:], hT[:, s])
                    nc.vector.tensor_copy(zT_sb[:, s], pzT_h[hh][:])
                hT_new = hst.tile([128, 512], F32R, tag="h", name=f"h{t}")
                for hh in range(2):
                    s = slice(256 * hh, 256 * hh + 256)
                    d = tmpp.tile([128, 256], F32, tag="d", name=f"d{t}_{hh}")
                    nc.vector.scalar_tensor_tensor(d[:], zT_sb[:, s], 1.0,
                                                   pnT_h[hh][:],
                                                   OP.subtract, OP.mult)
                    nc.vector.tensor_sub(hT_new[:, s], u_t[:, s], d[:])
                hT = hT_new

            # ================= encoder =================
            with tc.tile_pool(name="wenc", bufs=1) as wenc:
                wih = wenc.tile([128, 4, 3 * H], F32R)
                for k in range(4):
                    nc.sync.dma_start(wih[:, k, :], wih_d[k])
                whh = wenc.tile([128, 8, 3 * H], F32R)
                for k in range(8):
                    nc.sync.dma_start(whh[:, k, :], whh_d[k])
                xt_tiles = {}
                for t in range(min(5, n_steps)):
                    xt_tiles[t] = xts.tile([128, 4, BL], F32R, tag="x", name=f"xt{t}")
                    nc.sync.dma_start(xt_tiles[t][:], xT_d[t])
                cur = {}
                cur["pz"] = alloc_seed_pair("pz", 0, True)
                cur["pr"] = alloc_seed_pair("pr", 0, True)
                emit_gi_zr(cur, xt_tiles[0])
                cur["pin"] = alloc_seed_pair("pin", 0, True)
                cur["phn"] = alloc_seed_pair("phn", 0, True)
                emit_gi_in(cur, xt_tiles[0])
                for t in range(n_steps):
                    if t + 5 < n_steps:
                        xt_tiles[t + 5] = xts.tile([128, 4, BL], F32R, tag="x",
                                                   name=f"xt{t+5}")
                        nc.sync.dma_start(xt_tiles[t + 5][:], xT_d[t + 5])
                    emit_gh(cur, whh, (("pz", 1024), ("pr", 0)))
                    if "pin" not in cur:
                        # late-seed in/hn so 4 psum banks stay free through
                        # the z/r half of the step (fewer slot-wait stalls)
                        cur["pin"] = alloc_seed_pair("pin", t, True)
                        cur["phn"] = alloc_seed_pair("phn", t, True)
                        emit_gi_in(cur, xt_tiles[t])
                    emit_gh(cur, whh, (("phn", 2048),))
                    g = cur
                    nxt = {}
                    if t + 1 < n_steps:
                        xt_next = xt_tiles[t + 1]

                        def filler(nxt=nxt, xt_next=xt_next, t=t):
                            nxt["pz"] = alloc_seed_pair("pz", t + 1, True)
                            nxt["pr"] = alloc_seed_pair("pr", t + 1, True)
                            emit_gi_zr(nxt, xt_next)
                    else:
                        filler = None
                    step_tail(t, g, filler)
                    cur = nxt
                    xt_tiles.pop(t, None)

            # ================= decoder =================
            with tc.tile_pool(name="wdec", bufs=1) as wdec:
                wcb = wdec.tile([128, 8, 4 * H], F32R)
                for k in range(8):
                    nc.sync.dma_start(wcb[:, k, :], wcb_d[k])
                wz = wdec.tile([128, 8, I], F32R)
                for k in range(8):
                    nc.sync.dma_start(wz[:, k, :], wz_d[k])
                cur = {}
                for nm in ("pz", "pr"):
                    cur[nm] = alloc_seed_pair(nm, 1000, False)
                for t in range(n_steps):
                    emit_gh(cur, wcb, (("pz", 1024), ("pr", 0)))
                    cur["phn"] = alloc_seed_pair("phn", 1000 + t, False)
                    cur["pin"] = alloc_seed_pair("pin", 1000 + t, False)
                    emit_gh(cur, wcb, (("phn", 3072), ("pin", 2048)))
                    g = cur
                    hT_entry = hT
                    nxt = {}
                    if t + 1 < n_steps:

                        def filler(nxt=nxt, hT_entry=hT_entry, t=t):
                            nxt["pz"] = alloc_seed_pair("pz", 1001 + t, False)
                            nxt["pr"] = alloc_seed_pair("pr", 1001 + t, False)
                            if t >= 1:
                                emit_zfill(hT_entry, t - 1)
                    else:

                        def filler(hT_entry=hT_entry, t=t):
                            emit_zfill(hT_entry, t - 1)
                    step_tail(1000 + t, g, filler)
                    cur = nxt
                # final z output from last hidden state
                emit_zfill(hT, n_steps - 1)
    return nc


# ---------------------------------------------------------------- host side
def _prep_shared(enc_Wih, enc_Whh, enc_bih, enc_bhh,
                 dec_Wih, dec_Whh, dec_bih, dec_bhh, Wz, bz):
    f32 = np.float32
    wihT = np.ascontiguousarray(enc_Wih.T, dtype=f32).reshape(4, 128, 3 * H)
    whhT = np.ascontiguousarray(enc_Whh.T, dtype=f32).reshape(8, 128, 3 * H)
    wcomb = np.concatenate(
        [dec_Wih[: 2 * H] + dec_Whh[: 2 * H], dec_Wih[2 * H :], dec_Whh[2 * H :]], 0
    )
    wcombT = np.ascontiguousarray(wcomb.T, dtype=f32).reshape(8, 128, 4 * H)
    wzT = np.ascontiguousarray(Wz.T, dtype=f32).reshape(8, 128, I)
    # Seed-bias rows: seat (gate, half) -> (row in {0,32,64,96}, col block).
    # enc uses cols 0:1024 (z/r at 0:512, in/hn at 512:1024); dec cols 1024:2048.
    brows = np.zeros((128, 2048), f32)
    for cbase, bih, bhh in ((0, enc_bih, enc_bhh), (1024, dec_bih, dec_bhh)):
        be = bih + bhh
        brows[0, cbase : cbase + 512] = be[H : H + 512]          # z half 0
        brows[32, cbase : cbase + 512] = be[H + 512 : 2 * H]     # z half 1
        brows[64, cbase : cbase + 512] = be[0:512]               # r half 0
        brows[96, cbase : cbase + 512] = be[512:H]               # r half 1
        brows[0, cbase + 512 : cbase + 1024] = bih[2 * H : 2 * H + 512]
        brows[32, cbase + 512 : cbase + 1024] = bih[2 * H + 512 :]
        brows[64, cbase + 512 : cbase + 1024] = bhh[2 * H : 2 * H + 512]
        brows[96, cbase + 512 : cbase + 1024] = bhh[2 * H + 512 :]
    ones = np.ones((128, 64), f32)
    iden = np.tile(np.eye(64, dtype=f32), (2, 1))
    h0T = np.full((128, 512), 0.1, f32)
    return {
        "wihT": wihT, "whhT": whhT, "wcombT": wcombT, "wzT": wzT,
        "brows": brows, "bzt": np.tile(np.asarray(bz, f32)[None, :], (64, 1)),
        "ones": ones, "iden": iden, "h0T": h0T,
    }


def kernel(x, enc_Wih, enc_Whh, enc_bih, enc_bhh,
           dec_Wih, dec_Whh, dec_bih, dec_bhh, Wz, bz, n_steps=T):
    x = np.asarray(x, np.float32)
    shared = _prep_shared(enc_Wih, enc_Whh, enc_bih, enc_bhh,
                          dec_Wih, dec_Whh, dec_bih, dec_bhh, Wz, bz)
    in_maps = []
    for c in range(N_CORES):
        xc = x[c * BL : (c + 1) * BL, :n_steps]  # [BL, n_steps, I]
        # [T, klo, kx, b] so each step's tile loads in a single DMA
        xT = np.ascontiguousarray(
            xc.transpose(1, 2, 0).reshape(n_steps, 4, 128, BL).transpose(0, 2, 1, 3)
        )
        in_maps.append({"xT": xT, **shared})

    nc = build_nc(n_steps)
    _split_multi_waits(nc)

    trace = bool(int(os.environ.get("GRU_TRACE", "0")))
    if trace:
        _install_ntff_hook()
    res = bass_utils.run_bass_kernel_spmd(
        nc, in_maps, core_ids=list(range(N_CORES)), trace=trace
    )
    if trace and res.exec_time_ns is not None:
        print(f"HW exec time: {res.exec_time_ns} ns")
    out = np.concatenate([res.results[c]["z"] for c in range(N_CORES)], axis=0)
    return out



# revision 36
# speedup vs baseline: 1.0189x; 1.0189x over previous
"""GRU autoencoder Trainium2 kernel.

Data-parallel over batch: 8 cores x 64 rows. Per core, the recurrence keeps
the hidden state TRANSPOSED in SBUF (hT[klo, 64*khi+b] = h[b, 128*khi+klo])
so it can be the stationary matmul operand directly. Gates are computed as
h @ W.T with fp32r matmuls (M=64, N=512) accumulating in PSUM on top of K=1
bias-seed matmuls; the seeds sit in four distinct PE row groups so adjacent
seeds stream concurrently through the array. The z/n gates are folded onto
both partition halves so their transposes back to hT layout (transpose-mode
identity matmuls) run as concurrent row-group pairs into separate PSUM
banks. The hidden update uses hT' = zT*hT - (zT-1)*nT with the zT terms
computed while the tanh path is still in flight. Next-step input-gate
matmuls and decoder z-outputs are emitted during the elementwise tail so
the PE never drains; in/hn psums are seeded mid-step to cut PSUM-bank
pressure.
"""
import os
import sys
import types

import numpy as np

import concourse.bass as bass
import concourse.mybir as mybir
import concourse.tile as tile
from concourse import bass_utils

F32 = mybir.dt.float32
F32R = mybir.dt.float32r
AF = mybir.ActivationFunctionType
OP = mybir.AluOpType

N_CORES = 8
B, T, I, H = 512, 128, 512, 1024
BL = B // N_CORES  # 64


# ---------------------------------------------------------------- fixups
_CTRL_OPCODES = {"Drain", "NoOp", "EventSemaphore", "AllEngineBarrier", "Halt"}


def _split_multi_waits(nc, max_waits=1):
    """This walrus build allows only one sync-wait per instruction; hoist
    excess waits onto preceding NoOps (same engine, so semantics hold)."""
    for f in nc.m.functions:
        for blk in f.blocks:
            insts = blk.instructions
            if not any(
                i.sync_info is not None
                and i.sync_info.on_wait
                and len(i.sync_info.on_wait) > max_waits
                for i in insts
            ):
                continue
            new = []
            for inst in insts:
                si = inst.sync_info
                if si is not None and si.on_wait and len(si.on_wait) > max_waits:
                    waits = list(si.on_wait)
                    extra, keep = waits[:-max_waits], waits[-max_waits:]
                    for cs in range(0, len(extra), max_waits):
                        nop = mybir.InstNoOp(
                            name=nc.get_next_instruction_name(),
                            engine=inst.engine,
                            ins=[],
                            outs=[],
                            sync_info=mybir.SyncInfo(
                                on_wait=extra[cs : cs + max_waits], on_update=[]
                            ),
                        )
                        nc.register_instruction(nop)
                        new.append(nop)
                    si.on_wait = keep
                new.append(inst)
            insts[:] = new


def _install_ntff_hook():
    if "antenv.axon_hooks" in sys.modules:
        return True
    mod = types.ModuleType("antenv.axon_hooks")
    state = {"hook": None}
    mod.set_axon_ntff_profile_hook = lambda h: state.__setitem__("hook", h)
    mod.get_axon_ntff_profile_hook = lambda: state["hook"]
    sys.modules["antenv.axon_hooks"] = mod
    try:
        import antenv

        antenv.axon_hooks = mod
        from trn_agent_boot.trn_boot import _ntff_profile_via_ctypes

        hook = _ntff_profile_via_ctypes("/opt/axon/libaxon_pjrt.so")
        if hook is None:
            return False
        mod.set_axon_ntff_profile_hook(hook)
        return True
    except Exception:
        return False


# ---------------------------------------------------------------- program
def build_nc(n_steps=T):
    nc = bass.Bass("TRN2", target_bir_lowering=False, debug=False, num_devices=N_CORES)

    xT_d = nc.dram_tensor("xT", [n_steps, 4, 128, BL], F32R, kind="ExternalInput").ap()
    wih_d = nc.dram_tensor("wihT", [4, 128, 3 * H], F32R, kind="ExternalInput").ap()
    whh_d = nc.dram_tensor("whhT", [8, 128, 3 * H], F32R, kind="ExternalInput").ap()
    wcb_d = nc.dram_tensor("wcombT", [8, 128, 4 * H], F32R, kind="ExternalInput").ap()
    wz_d = nc.dram_tensor("wzT", [8, 128, I], F32R, kind="ExternalInput").ap()
    br_d = nc.dram_tensor("brows", [128, 2048], F32R, kind="ExternalInput").ap()
    bz_d = nc.dram_tensor("bzt", [64, I], F32, kind="ExternalInput").ap()
    on_d = nc.dram_tensor("ones", [128, 64], F32R, kind="ExternalInput").ap()
    id_d = nc.dram_tensor("iden", [128, 64], F32R, kind="ExternalInput").ap()
    h0_d = nc.dram_tensor("h0T", [128, 512], F32R, kind="ExternalInput").ap()
    z_d = nc.dram_tensor("z", [BL, n_steps, I], F32, kind="ExternalOutput").ap()

    with tile.TileContext(nc) as tc:
        with (
            tc.tile_pool(name="cst", bufs=1) as cst,
            tc.tile_pool(name="hst", bufs=3) as hst,
            tc.tile_pool(name="xts", bufs=5) as xts,
            tc.tile_pool(name="gsb", bufs=2) as gsb,
            tc.tile_pool(name="tmp", bufs=2) as tmpp,
            tc.tile_pool(name="zo", bufs=2) as zop,
            tc.tile_pool(name="ps", bufs=8, space="PSUM") as ps,
        ):
            brows = cst.tile([128, 2048], F32R)
            nc.sync.dma_start(brows[:], br_d[:])
            bzt = cst.tile([64, I], F32)
            nc.sync.dma_start(bzt[:], bz_d[:])
            ones = cst.tile([128, 64], F32R)
            nc.sync.dma_start(ones[:], on_d[:])
            iden = cst.tile([128, 64], F32R)
            nc.sync.dma_start(iden[:], id_d[:])
            hT = hst.tile([128, 512], F32R, tag="h")
            nc.sync.dma_start(hT[:], h0_d[:])

            def seed(pt, brow_ap, one_ap, bp):
                nc.tensor.matmul(pt[:], one_ap, brow_ap, start=True, stop=False,
                                 tile_position=(bp, 0))

            def alloc_seed_pair(nm, t, enc):
                """Allocate+bias-seed one gate pair (2 psum tiles).

                The two seeds of a pair sit in different PE row groups (and
                z/in vs r/hn pairs in different groups again), so adjacent
                seed matmuls stream concurrently in the array."""
                cbase = 0 if enc else 1024
                rows, c0 = {
                    "pz": ((0, 32), cbase), "pr": ((64, 96), cbase),
                    "pin": ((0, 32), cbase + 512), "phn": ((64, 96), cbase + 512),
                }[nm]
                tiles = [ps.tile([64, 512], F32, tag="ps", name=f"{nm}{i}_{t}")
                         for i in range(2)]
                for nt in range(2):
                    row = rows[nt]
                    seed(tiles[nt], brows[row : row + 1, c0 : c0 + 512],
                         ones[row : row + 1, :], row)
                return tiles

            def emit_gi_zr(g, xt):
                for tiles, c0 in ((g["pz"], 1024), (g["pr"], 0)):
                    for nt in range(2):
                        c = c0 + 512 * nt
                        for k in range(4):
                            nc.tensor.matmul(
                                tiles[nt][:], xt[:, k, :], wih[:, k, c : c + 512],
                                start=False, stop=False,
                            )

            def emit_gi_in(g, xt):
                for nt in range(2):
                    c = 2048 + 512 * nt
                    for k in range(4):
                        nc.tensor.matmul(
                            g["pin"][nt][:], xt[:, k, :], wih[:, k, c : c + 512],
                            start=False, stop=(k == 3),
                        )

            def emit_gh(g, w, cols):
                """Recurrent gate matmuls reading hT: order z, r, (in), hn."""
                for nm, c0 in cols:
                    for nt in range(2):
                        c = c0 + 512 * nt
                        for k in range(8):
                            nc.tensor.matmul(
                                g[nm][nt][:],
                                hT[:, 64 * k : 64 * k + 64],
                                w[:, k, c : c + 512],
                                start=False, stop=(k == 7),
                            )

            def emit_zfill(src_hT, t_out):
                pzo = ps.tile([64, 512], F32, tag="ps", name=f"pzo{t_out}")
                for j in range(8):
                    nc.tensor.matmul(
                        pzo[:], src_hT[:, 64 * j : 64 * j + 64], wz[:, j, :],
                        start=(j == 0), stop=(j == 7),
                    )
                zo_sb = zop.tile([64, 512], F32, tag="zo", name=f"zo{t_out}")
                nc.vector.tensor_add(zo_sb[:], pzo[:], bzt[:])
                nc.sync.dma_start(z_d[:, t_out, :], zo_sb[:])

            def step_tail(t, g, filler):
                """sigmoids, transposes, n-chain, h-update; filler() emits
                next-step PE work between zT and nT transposes."""
                nonlocal hT
                # z folded onto both partition halves so its transposes run
                # as concurrent row-group pairs
                z_f = gsb.tile([128, 512], F32R, tag="z", name=f"z{t}")
                for nt in range(2):
                    nc.scalar.activation(z_f[64 * nt : 64 * nt + 64, :],
                                         g["pz"][nt][:], AF.Sigmoid)
                # two half tiles in separate banks: the row-group pairs may
                # not write one PSUM bank concurrently
                pzT_h = [ps.tile([128, 256], F32R, tag="ps", name=f"pzT{t}_{i}")
                         for i in range(2)]
                for j in range(4):
                    nc.tensor.transpose(
                        pzT_h[0][:, 64 * j : 64 * j + 64],
                        z_f[0:64, 128 * j : 128 * j + 128],
                        iden[0:64, :],
                    )
                    nc.tensor.transpose(
                        pzT_h[1][:, 64 * j : 64 * j + 64],
                        z_f[64:128, 128 * j : 128 * j + 128],
                        iden[64:128, :],
                    )
                r_sb = gsb.tile([64, 1024], F32, tag="r", name=f"r{t}")
                for nt in range(2):
                    nc.scalar.activation(r_sb[:, 512 * nt : 512 * nt + 512],
                                         g["pr"][nt][:], AF.Sigmoid)

                if filler is not None:
                    filler()

                # u = zT * hT can run while the n-path (tanh/transpose) works
                u_t = tmpp.tile([128, 512], F32, tag="u", name=f"u{t}")

                # n = tanh(in + r*hn), folded onto partition halves like z so
                # the transposes run as concurrent row-group pairs
                n_f = gsb.tile([128, 512], F32R, tag="n", name=f"n{t}")
                pnT_h = [ps.tile([128, 256], F32R, tag="ps", name=f"pnT{t}_{i}")
                         for i in range(2)]
                for nt in range(2):
                    t1 = tmpp.tile([64, 512], F32, tag="t1", name=f"t1_{t}_{nt}")
                    nc.vector.tensor_mul(t1[:], r_sb[:, 512 * nt : 512 * nt + 512],
                                         g["phn"][nt][:])
                    nc.vector.tensor_add(t1[:], t1[:], g["pin"][nt][:])
                    nc.scalar.activation(n_f[64 * nt : 64 * nt + 64, :],
                                         t1[:], AF.Tanh)
                for j in range(4):
                    nc.tensor.transpose(
                        pnT_h[0][:, 64 * j : 64 * j + 64],
                        n_f[0:64, 128 * j : 128 * j + 128],
                        iden[0:64, :],
                    )
                    nc.tensor.transpose(
                        pnT_h[1][:, 64 * j : 64 * j + 64],
                        n_f[64:128, 128 * j : 128 * j + 128],
                        iden[64:128, :],
                    )

                # hT' = zT*hT + (1-zT)*nT as u - (zT-1)*nT; u and the zT sbuf
                # stage only need zT, so they overlap the tanh/nT path.
                zT_sb = tmpp.tile([128, 512], F32, tag="zts", name=f"zts{t}")
                for hh in range(2):
                    s = slice(256 * hh, 256 * hh + 256)
                    nc.vector.tensor_mul(u_t[:, s], pzT_h[hh][:], hT[:, s])
                    nc.vector.tensor_copy(zT_sb[:, s], pzT_h[hh][:])
                hT_new = hst.tile([128, 512], F32R, tag="h", name=f"h{t}")
                for hh in range(2):
                    s = slice(256 * hh, 256 * hh + 256)
                    d = tmpp.tile([128, 256], F32, tag="d", name=f"d{t}_{hh}")
                    nc.vector.scalar_tensor_tensor(d[:], zT_sb[:, s], 1.0,
                                                   pnT_h[hh][:],
                                                   OP.subtract, OP.mult)
                    nc.vector.tensor_sub(hT_new[:, s], u_t[:, s], d[:])
                hT = hT_new

            # ================= encoder =================
            with tc.tile_pool(name="wenc", bufs=1) as wenc:
                wih = wenc.tile([128, 4, 3 * H], F32R)
                for k in range(4):
                    nc.sync.dma_start(wih[:, k, :], wih_d[k])
                whh = wenc.tile([128, 8, 3 * H], F32R)
                for k in range(8):
                    nc.sync.dma_start(whh[:, k, :], whh_d[k])
                xt_tiles = {}
                for t in range(min(5, n_steps)):
                    xt_tiles[t] = xts.tile([128, 4, BL], F32R, tag="x", name=f"xt{t}")
                    for k in range(4):
                        nc.sync.dma_start(xt_tiles[t][:, k, :], xT_d[t, k])
                cur = {}
                cur["pz"] = alloc_seed_pair("pz", 0, True)
                cur["pr"] = alloc_seed_pair("pr", 0, True)
                emit_gi_zr(cur, xt_tiles[0])
                cur["pin"] = alloc_seed_pair("pin", 0, True)
                cur["phn"] = alloc_seed_pair("phn", 0, True)
                emit_gi_in(cur, xt_tiles[0])
                for t in range(n_steps):
                    if t + 5 < n_steps:
                        xt_tiles[t + 5] = xts.tile([128, 4, BL], F32R, tag="x",
                                                   name=f"xt{t+5}")
                        for k in range(4):
                            nc.sync.dma_start(xt_tiles[t + 5][:, k, :], xT_d[t + 5, k])
                    emit_gh(cur, whh, (("pz", 1024), ("pr", 0)))
                    if "pin" not in cur:
                        # late-seed in/hn so 4 psum banks stay free through
                        # the z/r half of the step (fewer slot-wait stalls)
                        cur["pin"] = alloc_seed_pair("pin", t, True)
                        cur["phn"] = alloc_seed_pair("phn", t, True)
                        emit_gi_in(cur, xt_tiles[t])
                    emit_gh(cur, whh, (("phn", 2048),))
                    g = cur
                    nxt = {}
                    if t + 1 < n_steps:
                        xt_next = xt_tiles[t + 1]

                        def filler(nxt=nxt, xt_next=xt_next, t=t):
                            nxt["pz"] = alloc_seed_pair("pz", t + 1, True)
                            nxt["pr"] = alloc_seed_pair("pr", t + 1, True)
                            emit_gi_zr(nxt, xt_next)
                    else:
                        filler = None
                    step_tail(t, g, filler)
                    cur = nxt
                    xt_tiles.pop(t, None)

            # ================= decoder =================
            with tc.tile_pool(name="wdec", bufs=1) as wdec:
                wcb = wdec.tile([128, 8, 4 * H], F32R)
                for k in range(8):
                    nc.sync.dma_start(wcb[:, k, :], wcb_d[k])
                wz = wdec.tile([128, 8, I], F32R)
                for k in range(8):
                    nc.sync.dma_start(wz[:, k, :], wz_d[k])
                cur = {}
                for nm in ("pz", "pr"):
                    cur[nm] = alloc_seed_pair(nm, 1000, False)
                for t in range(n_steps):
                    emit_gh(cur, wcb, (("pz", 1024), ("pr", 0)))
                    cur["phn"] = alloc_seed_pair("phn", 1000 + t, False)
                    cur["pin"] = alloc_seed_pair("pin", 1000 + t, False)
                    emit_gh(cur, wcb, (("phn", 3072), ("pin", 2048)))
                    g = cur
                    hT_entry = hT
                    nxt = {}
                    if t + 1 < n_steps:

                        def filler(nxt=nxt, hT_entry=hT_entry, t=t):
                            nxt["pz"] = alloc_seed_pair("pz", 1001 + t, False)
                            nxt["pr"] = alloc_seed_pair("pr", 1001 + t, False)
                            if t >= 1:
                                emit_zfill(hT_entry, t - 1)
                    else:

                        def filler(hT_entry=hT_entry, t=t):
                            emit_zfill(hT_entry, t - 1)
                    step_tail(1000 + t, g, filler)
                    cur = nxt
                # final z output from last hidden state
                emit_zfill(hT, n_steps - 1)
    return nc


# ---------------------------------------------------------------- host side
def _prep_shared(enc_Wih, enc_Whh, enc_bih, enc_bhh,
                 dec_Wih, dec_Whh, dec_bih, dec_bhh, Wz, bz):
    f32 = np.float32
    wihT = np.ascontiguousarray(enc_Wih.T, dtype=f32).reshape(4, 128, 3 * H)
    whhT = np.ascontiguousarray(enc_Whh.T, dtype=f32).reshape(8, 128, 3 * H)
    wcomb = np.concatenate(
        [dec_Wih[: 2 * H] + dec_Whh[: 2 * H], dec_Wih[2 * H :], dec_Whh[2 * H :]], 0
    )
    wcombT = np.ascontiguousarray(wcomb.T, dtype=f32).reshape(8, 128, 4 * H)
    wzT = np.ascontiguousarray(Wz.T, dtype=f32).reshape(8, 128, I)
    # Seed-bias rows: seat (gate, half) -> (row in {0,32,64,96}, col block).
    # enc uses cols 0:1024 (z/r at 0:512, in/hn at 512:1024); dec cols 1024:2048.
    brows = np.zeros((128, 2048), f32)
    for cbase, bih, bhh in ((0, enc_bih, enc_bhh), (1024, dec_bih, dec_bhh)):
        be = bih + bhh
        brows[0, cbase : cbase + 512] = be[H : H + 512]          # z half 0
        brows[32, cbase : cbase + 512] = be[H + 512 : 2 * H]     # z half 1
        brows[64, cbase : cbase + 512] = be[0:512]               # r half 0
        brows[96, cbase : cbase + 512] = be[512:H]               # r half 1
        brows[0, cbase + 512 : cbase + 1024] = bih[2 * H : 2 * H + 512]
        brows[32, cbase + 512 : cbase + 1024] = bih[2 * H + 512 :]
        brows[64, cbase + 512 : cbase + 1024] = bhh[2 * H : 2 * H + 512]
        brows[96, cbase + 512 : cbase + 1024] = bhh[2 * H + 512 :]
    ones = np.ones((128, 64), f32)
    iden = np.tile(np.eye(64, dtype=f32), (2, 1))
    h0T = np.full((128, 512), 0.1, f32)
    return {
        "wihT": wihT, "whhT": whhT, "wcombT": wcombT, "wzT": wzT,
        "brows": brows, "bzt": np.tile(np.asarray(bz, f32)[None, :], (64, 1)),
        "ones": ones, "iden": iden, "h0T": h0T,
    }


def kernel(x, enc_Wih, enc_Whh, enc_bih, enc_bhh,
           dec_Wih, dec_Whh, dec_bih, dec_bhh, Wz, bz, n_steps=T):
    x = np.asarray(x, np.float32)
    shared = _prep_shared(enc_Wih, enc_Whh, enc_bih, enc_bhh,
                          dec_Wih, dec_Whh, dec_bih, dec_bhh, Wz, bz)
    in_maps = []
    for c in range(N_CORES):
        xc = x[c * BL : (c + 1) * BL, :n_steps]  # [BL, n_steps, I]
        xT = np.ascontiguousarray(xc.transpose(1, 2, 0)).reshape(n_steps, 4, 128, BL)
        in_maps.append({"xT": xT, **shared})

    nc = build_nc(n_steps)
    _split_multi_waits(nc)

    trace = bool(int(os.environ.get("GRU_TRACE", "0")))
    if trace:
        _install_ntff_hook()
    res = bass_utils.run_bass_kernel_spmd(
        nc, in_maps, core_ids=list(range(N_CORES)), trace=trace
    )
    if trace and res.exec_time_ns is not None:
        print(f"HW exec time: {res.exec_time_ns} ns")
    out = np.concatenate([res.results[c]["z"] for c in range(N_CORES)], axis=0)
    return out

